# revision 25
# baseline (speedup 1.0000x reference)
"""Multi-head attention (B=2, S=2048, E=1024, H=16) on 8 trn2 NeuronCores.

Sharding: tensor-parallel over heads (2 heads per core).  Each core computes
q/k/v for its 2 heads from the full x, runs attention, and produces a partial
output projection (row-split w_proj); the host sums the 8 partials (the
"all-reduce" of the row-split projection) and adds b_proj.

Device dataflow is feature-major (transposed activations) end to end:
  xT [E, B*S] (bf16)  --(lhsT=W_loc)-->  qT/kT/vT [128, S]  (128 = 2 hd x 64)
  scoresT [t, s_q] = kT_h.T-part @ qT_h  (contraction over d_h=64; the two
    heads go to disjoint PE row-groups via tile_position and land side by
    side in one 2-bank psum tile)
  attnT = exp(scoresT) in bf16, one 1024-wide ACT op per t-chunk
    (1/sqrt(d) scale folded into w_q on host; max-subtraction skipped --
    scores are ~N(0,1), exp can't overflow)
  outT_unnorm[65, s_q] accum over t-chunks = [v | ones].T @ attnT
    (row 64 = softmax denominators, for free)
  per-q-tile: reciprocal_approx_fast on the denominator row, DRAM-bounced
    stride-0 broadcast, DVE multiply, then that q-tile's slice of the output
    projection -- everything pipelines behind the next q-tile's attention.
  Phase A (qkv projection) of batch b+1 and its v-transposes are emitted
  interleaved into batch b's attention so the PE never drains between phases.
"""

import ml_dtypes
import numpy as np

import concourse.bass as bass
import concourse.mybir as mybir
import concourse.tile as tile
from concourse import bacc
from concourse.bass_utils import run_bass_kernel_spmd
from concourse.masks import make_identity

F32 = mybir.dt.float32
BF16 = mybir.dt.bfloat16
NPBF16 = ml_dtypes.bfloat16

E = 1024
NH = 16
DH = 64
NCORES = 8
HPC = NH // NCORES  # heads per core = 2
LF = HPC * DH  # local features per core = 128
NCHUNK = E // 128  # contraction chunks for the qkv projection = 8


def build_nc(B=2, S=2048):
    ST = min(512, S // 2)  # free-dim tile
    SH = S // 2  # s-half processed per xT load
    NST = SH // ST  # s-tiles per half
    NTT = S // 128  # 128-row t-chunks per batch
    NQ = S // ST  # q-tiles per batch
    BS = B * S

    nc = bacc.Bacc("TRN2")
    xT = nc.dram_tensor("xT", [E, BS], BF16, kind="ExternalInput")
    wq = nc.dram_tensor("wq", [E, LF], BF16, kind="ExternalInput")
    wk = nc.dram_tensor("wk", [E, LF], BF16, kind="ExternalInput")
    wv = nc.dram_tensor("wv", [E, LF], BF16, kind="ExternalInput")
    bq = nc.dram_tensor("bq", [LF, 1], F32, kind="ExternalInput")
    bk = nc.dram_tensor("bk", [LF, 1], F32, kind="ExternalInput")
    bv = nc.dram_tensor("bv", [LF, 1], F32, kind="ExternalInput")
    wp = nc.dram_tensor("wp", [LF, E], BF16, kind="ExternalInput")
    ones16_d = nc.dram_tensor("ones16", [128, DH], BF16, kind="ExternalInput")
    y = nc.dram_tensor("y", [BS, E], F32, kind="ExternalOutput")

    mm = nc.tensor.matmul

    with tile.TileContext(nc) as tc:
        with (
            tc.tile_pool(name="consts", bufs=1) as consts,
            tc.tile_pool(name="xpool", bufs=3) as xpool,
            tc.tile_pool(name="acts", bufs=2) as acts,
            tc.tile_pool(name="vtp", bufs=1) as vtp,
            tc.tile_pool(name="vap", bufs=2) as vap,
            tc.tile_pool(name="attp", bufs=6) as attp,
            tc.tile_pool(name="npool", bufs=3) as npool,
            tc.tile_pool(name="ypool", bufs=4) as ypool,
            tc.tile_pool(name="psA", bufs=2, space="PSUM") as psA,
            tc.tile_pool(name="psS", bufs=2, space="PSUM") as psS,
            tc.tile_pool(name="psO", bufs=2, space="PSUM") as psO,
            tc.tile_pool(name="dramp", bufs=2, space="DRAM") as dramp,
        ):
            # ---- constants ----
            wq_sb = consts.tile([128, NCHUNK, LF], BF16, tag="wq")
            wk_sb = consts.tile([128, NCHUNK, LF], BF16, tag="wk")
            wv_sb = consts.tile([128, NCHUNK, LF], BF16, tag="wv")
            nc.sync.dma_start(out=wq_sb, in_=wq.rearrange("(c p) n -> p c n", p=128))
            nc.sync.dma_start(out=wk_sb, in_=wk.rearrange("(c p) n -> p c n", p=128))
            nc.sync.dma_start(out=wv_sb, in_=wv.rearrange("(c p) n -> p c n", p=128))
            wp_sb = consts.tile([LF, E], BF16, tag="wp")
            nc.sync.dma_start(out=wp_sb, in_=wp[:, :])
            bq_sb = consts.tile([LF, 1], F32, tag="bq")
            bk_sb = consts.tile([LF, 1], F32, tag="bk")
            bv_sb = consts.tile([LF, 1], F32, tag="bv")
            nc.sync.dma_start(out=bq_sb, in_=bq[:, :])
            nc.sync.dma_start(out=bk_sb, in_=bk[:, :])
            nc.sync.dma_start(out=bv_sb, in_=bv[:, :])
            ident = consts.tile([128, 128], BF16, tag="ident")
            make_identity(nc, ident)

            xT_r = xT.rearrange("(c p) s -> p c s", p=128)

            # per-batch state, filled lazily as phases are emitted
            qTs, kTs, vTs, vaugs, aoTs, u_alls, xts = {}, {}, {}, {}, {}, {}, {}

            def emit_A_group(b, sh, which):
                """One (s-half, tensor) block of the qkv projection."""
                if b not in qTs:
                    qTs[b] = acts.tile([128, S], BF16, tag="qT", name=f"qT{b}")
                    kTs[b] = acts.tile([128, S], BF16, tag="kT", name=f"kT{b}")
                    vTs[b] = vtp.tile([128, S], BF16, tag="vT", name=f"vT{b}")
                dst, w_sb, b_sb = {
                    "q": (qTs[b], wq_sb, bq_sb),
                    "k": (kTs[b], wk_sb, bk_sb),
                    "v": (vTs[b], wv_sb, bv_sb),
                }[which]
                if (b, sh) not in xts:
                    xt_new = xpool.tile(
                        [128, NCHUNK, SH], BF16, tag="xt", name=f"xt{b}{sh}"
                    )
                    s0 = b * S + sh * SH
                    nc.sync.dma_start(out=xt_new, in_=xT_r[:, :, s0 : s0 + SH])
                    xts[(b, sh)] = xt_new
                xt_sb = xts[(b, sh)]
                pss = []
                for st in range(NST):
                    ps = psA.tile([128, ST], F32, tag="psA", name=f"ps{st}")
                    pss.append(ps)
                for c in range(NCHUNK):
                    for st in range(NST):
                        mm(
                            pss[st],
                            lhsT=w_sb[:, c, :],
                            rhs=xt_sb[:, c, st * ST : (st + 1) * ST],
                            start=(c == 0),
                            stop=(c == NCHUNK - 1),
                        )
                for st in range(NST):
                    g0 = sh * SH + st * ST
                    nc.vector.tensor_scalar_add(dst[:, g0 : g0 + ST], pss[st], b_sb)

            def emit_transposes(b):
                """vT -> v_aug [t, (v_h | ones)] via PE transpose."""
                v_aug = vap.tile(
                    [128, NTT, 2 * (DH + 1)], BF16, tag="vaug", name=f"vaug{b}"
                )
                vaugs[b] = v_aug
                ones_col = ones16_d[:, 0:NTT].unsqueeze(2)
                nc.sync.dma_start(out=v_aug[:, :, DH : DH + 1], in_=ones_col)
                nc.sync.dma_start(
                    out=v_aug[:, :, 2 * DH + 1 : 2 * DH + 2], in_=ones_col
                )
                vT = vTs[b]
                for tt in range(NTT):
                    for h in range(HPC):
                        pst = psO.tile([128, ST], BF16, tag="psO", name="pst")
                        nc.tensor.matmul(
                            pst[:, 0:DH],
                            lhsT=vT[h * DH : (h + 1) * DH, tt * 128 : (tt + 1) * 128],
                            rhs=ident[h * DH : (h + 1) * DH, h * DH : (h + 1) * DH],
                            is_transpose=True,
                        )
                        nc.vector.tensor_copy(
                            v_aug[:, tt, h * (DH + 1) : h * (DH + 1) + DH],
                            pst[:, 0:DH],
                        )

            def emit_attention_qt(b, qt):
                """Attention + normalization + output projection for one
                512-wide q-tile."""
                if b not in aoTs:
                    aoTs[b] = acts.tile([128, S], BF16, tag="aoT", name=f"aoT{b}")
                    u_alls[b] = npool.tile(
                        [DH, HPC * NQ, ST], F32, tag="u_all", name=f"u_all{b}"
                    )
                qT, kT, v_aug, aoT = qTs[b], kTs[b], vaugs[b], aoTs[b]
                u_all = u_alls[b]
                qsl = slice(qt * ST, (qt + 1) * ST)
                out_ps = []
                for h in range(HPC):
                    o_ps = psO.tile([128, ST], F32, tag="psO", name=f"psO_{h}")
                    out_ps.append(o_ps)
                for tt in range(NTT):
                    tsl = slice(tt * 128, (tt + 1) * 128)
                    ps_s = psS.tile([128, HPC * ST], F32, tag="psS")
                    a = attp.tile([128, HPC * ST], BF16, tag="att")
                    for h in range(HPC):
                        hsl = slice(h * DH, (h + 1) * DH)
                        mm(
                            ps_s[:, h * ST : (h + 1) * ST],
                            lhsT=kT[hsl, tsl],
                            rhs=qT[hsl, qsl],
                            start=True,
                            stop=True,
                            tile_position=(h * DH, 0),
                        )
                    nc.scalar.activation(a, ps_s, mybir.ActivationFunctionType.Exp)
                    for h in range(HPC):
                        mm(
                            out_ps[h][0 : DH + 1, :],
                            lhsT=v_aug[:, tt, h * (DH + 1) : (h + 1) * (DH + 1)],
                            rhs=a[:, h * ST : (h + 1) * ST],
                            start=(tt == 0),
                            stop=(tt == NTT - 1),
                        )
                # normalize this q-tile (denominator row 64 of each psO)
                for h in range(HPC):
                    idx = qt * HPC + h
                    nc.vector.tensor_copy(u_all[:, idx, :], out_ps[h][0:DH, :])
                    # copy the sums row out first so the psum bank releases
                    # immediately -- the slow single-lane reciprocal then runs
                    # off the critical path
                    sums_sb = npool.tile([1, ST], F32, tag="sums")
                    nc.vector.tensor_copy(sums_sb, out_ps[h][DH : DH + 1, :])
                    rec = npool.tile([1, ST], F32, tag="rec")
                    nc.vector.reciprocal(rec, sums_sb)
                    bc_sb = npool.tile([DH, ST], F32, tag="bc")
                    nc.gpsimd.partition_broadcast(bc_sb, rec)
                    nc.vector.tensor_mul(
                        aoT[h * DH : (h + 1) * DH, qsl], u_all[:, idx, :], bc_sb
                    )
                # this q-tile's slice of the output projection
                for st in range(ST // 128):
                    s_loc = qt * ST + st * 128
                    r0 = b * S + s_loc
                    for eh in range(E // 512):
                        esl = slice(eh * 512, (eh + 1) * 512)
                        ps_y = psA.tile([128, 512], F32, tag="psA")
                        mm(
                            ps_y,
                            lhsT=aoT[:, s_loc : s_loc + 128],
                            rhs=wp_sb[:, esl],
                            start=True,
                            stop=True,
                        )
                        y_sb = ypool.tile([128, 512], F32, tag="y")
                        nc.vector.tensor_copy(y_sb, ps_y)
                        nc.sync.dma_start(out=y[r0 : r0 + 128, esl], in_=y_sb)

            # ---- emission schedule: batch 0's phase A, then per-q-tile
            # attention with the next batch's phase A interleaved ----
            INTERLEAVE = True
            for sh in range(2):
                for which in ("q", "k", "v"):
                    emit_A_group(0, sh, which)
            emit_transposes(0)
            items = [
                ("A", 0, "q"),
                ("A", 0, "k"),
                ("A", 0, "v"),
                ("A", 1, "v"),
                ("A", 1, "q"),
                ("A", 1, "k"),
            ]
            per_qt = -(-len(items) // NQ)  # ceil
            interleave = {
                qt: items[qt * per_qt : (qt + 1) * per_qt] for qt in range(NQ)
            }
            for b in range(B):
                if not INTERLEAVE and b > 0:
                    for sh in range(2):
                        for which in ("q", "k", "v"):
                            emit_A_group(b, sh, which)
                    emit_transposes(b)
                for qt in range(NQ):
                    emit_attention_qt(b, qt)
                    if INTERLEAVE and b + 1 < B:
                        for item in interleave.get(qt, []):
                            if item[0] == "A":
                                emit_A_group(b + 1, item[1], item[2])
                            else:
                                emit_transposes(b + 1)
                if INTERLEAVE and b + 1 < B:
                    emit_transposes(b + 1)

    nc.compile()
    return nc


_NC_CACHE = {}


def _get_nc(B, S):
    key = (B, S)
    if key not in _NC_CACHE:
        _NC_CACHE[key] = build_nc(B, S)
    return _NC_CACHE[key]


def make_in_maps(x, w_qkv, b_qkv, w_proj):
    B, S, _ = x.shape
    scale = DH**-0.5
    xT = np.ascontiguousarray(x.reshape(B * S, E).T).astype(NPBF16)
    in_maps = []
    for c in range(NCORES):
        cols = slice(c * LF, (c + 1) * LF)
        in_maps.append(
            {
                "xT": xT,
                "wq": (
                    np.ascontiguousarray(w_qkv[:, 0 * E : 1 * E][:, cols]) * scale
                ).astype(NPBF16),
                "wk": np.ascontiguousarray(w_qkv[:, 1 * E : 2 * E][:, cols]).astype(
                    NPBF16
                ),
                "wv": np.ascontiguousarray(w_qkv[:, 2 * E : 3 * E][:, cols]).astype(
                    NPBF16
                ),
                "bq": (b_qkv[0 * E : 1 * E][cols] * scale)
                .reshape(LF, 1)
                .astype(np.float32),
                "bk": b_qkv[1 * E : 2 * E][cols]
                .reshape(LF, 1)
                .astype(np.float32)
                .copy(),
                "bv": b_qkv[2 * E : 3 * E][cols]
                .reshape(LF, 1)
                .astype(np.float32)
                .copy(),
                "wp": np.ascontiguousarray(w_proj[cols, :]).astype(NPBF16),
                "ones16": np.ones((128, DH), dtype=NPBF16),
            }
        )
    return in_maps


def kernel_run(x, w_qkv, b_qkv, w_proj, b_proj, trace=False):
    x = np.asarray(x, dtype=np.float32)
    w_qkv = np.asarray(w_qkv, dtype=np.float32)
    b_qkv = np.asarray(b_qkv, dtype=np.float32)
    w_proj = np.asarray(w_proj, dtype=np.float32)
    b_proj = np.asarray(b_proj, dtype=np.float32)
    B, S, _ = x.shape
    nc = _get_nc(B, S)
    in_maps = make_in_maps(x, w_qkv, b_qkv, w_proj)
    res = run_bass_kernel_spmd(
        nc, in_maps, core_ids=list(range(NCORES)), trace=trace
    )
    y = res.results[0]["y"].astype(np.float64)
    for c in range(1, NCORES):
        y += res.results[c]["y"]
    y += b_proj[None, :]
    return y.astype(np.float32).reshape(B, S, E), res


def kernel(x, w_qkv, b_qkv, w_proj, b_proj):
    y, _ = kernel_run(x, w_qkv, b_qkv, w_proj, b_proj)
    return y


# revision 26
# speedup vs baseline: 1.0180x; 1.0180x over previous
"""Multi-head attention (B=2, S=2048, E=1024, H=16) on 8 trn2 NeuronCores.

Sharding: tensor-parallel over heads (2 heads per core).  Each core computes
q/k/v for its 2 heads from the full x, runs attention, and produces a partial
output projection (row-split w_proj); the host sums the 8 partials (the
"all-reduce" of the row-split projection) and adds b_proj.

Device dataflow is feature-major (transposed activations) end to end:
  xT [E, B*S] (bf16)  --(lhsT=W_loc)-->  qT/kT/vT [128, S]  (128 = 2 hd x 64)
  scoresT [t, s_q] = kT_h.T-part @ qT_h  (contraction over d_h=64; the two
    heads go to disjoint PE row-groups via tile_position and land side by
    side in one 2-bank psum tile)
  attnT = exp(scoresT) in bf16, one 1024-wide ACT op per t-chunk
    (1/sqrt(d) scale folded into w_q on host; max-subtraction skipped --
    scores are ~N(0,1), exp can't overflow)
  outT_unnorm[65, s_q] accum over t-chunks = [v | ones].T @ attnT
    (row 64 = softmax denominators, for free)
  per-q-tile: reciprocal_approx_fast on the denominator row, DRAM-bounced
    stride-0 broadcast, DVE multiply, then that q-tile's slice of the output
    projection -- everything pipelines behind the next q-tile's attention.
  Phase A (qkv projection) of batch b+1 and its v-transposes are emitted
  interleaved into batch b's attention so the PE never drains between phases.
"""

import ml_dtypes
import numpy as np

import concourse.bass as bass
import concourse.mybir as mybir
import concourse.tile as tile
from concourse import bacc
from concourse.bass_utils import run_bass_kernel_spmd
from concourse.masks import make_identity

F32 = mybir.dt.float32
BF16 = mybir.dt.bfloat16
NPBF16 = ml_dtypes.bfloat16

E = 1024
NH = 16
DH = 64
NCORES = 8
HPC = NH // NCORES  # heads per core = 2
LF = HPC * DH  # local features per core = 128
NCHUNK = E // 128  # contraction chunks for the qkv projection = 8


def build_nc(B=2, S=2048):
    ST = min(512, S // 2)  # free-dim tile
    SH = S // 2  # s-half processed per xT load
    NST = SH // ST  # s-tiles per half
    NTT = S // 128  # 128-row t-chunks per batch
    NQ = S // ST  # q-tiles per batch
    BS = B * S

    nc = bacc.Bacc("TRN2")
    xT = nc.dram_tensor("xT", [E, BS], BF16, kind="ExternalInput")
    wq = nc.dram_tensor("wq", [E, LF], BF16, kind="ExternalInput")
    wk = nc.dram_tensor("wk", [E, LF], BF16, kind="ExternalInput")
    wv = nc.dram_tensor("wv", [E, LF], BF16, kind="ExternalInput")
    bq = nc.dram_tensor("bq", [LF, 1], F32, kind="ExternalInput")
    bk = nc.dram_tensor("bk", [LF, 1], F32, kind="ExternalInput")
    bv = nc.dram_tensor("bv", [LF, 1], F32, kind="ExternalInput")
    wp = nc.dram_tensor("wp", [LF, E], BF16, kind="ExternalInput")
    ones16_d = nc.dram_tensor("ones16", [128, DH], BF16, kind="ExternalInput")
    y = nc.dram_tensor("y", [BS, E], F32, kind="ExternalOutput")

    mm = nc.tensor.matmul

    with tile.TileContext(nc) as tc:
        with (
            tc.tile_pool(name="consts", bufs=1) as consts,
            tc.tile_pool(name="xpool", bufs=3) as xpool,
            tc.tile_pool(name="acts", bufs=2) as acts,
            tc.tile_pool(name="vtp", bufs=1) as vtp,
            tc.tile_pool(name="vap", bufs=2) as vap,
            tc.tile_pool(name="attp", bufs=4) as attp,
            tc.tile_pool(name="npool", bufs=3) as npool,
            tc.tile_pool(name="ypool", bufs=4) as ypool,
            tc.tile_pool(name="psA", bufs=2, space="PSUM") as psA,
            tc.tile_pool(name="psS", bufs=2, space="PSUM") as psS,
            tc.tile_pool(name="psO", bufs=2, space="PSUM") as psO,
            tc.tile_pool(name="dramp", bufs=2, space="DRAM") as dramp,
        ):
            # ---- constants ----
            wq_sb = consts.tile([128, NCHUNK, LF], BF16, tag="wq")
            wk_sb = consts.tile([128, NCHUNK, LF], BF16, tag="wk")
            wv_sb = consts.tile([128, NCHUNK, LF], BF16, tag="wv")
            nc.sync.dma_start(out=wq_sb, in_=wq.rearrange("(c p) n -> p c n", p=128))
            nc.sync.dma_start(out=wk_sb, in_=wk.rearrange("(c p) n -> p c n", p=128))
            nc.sync.dma_start(out=wv_sb, in_=wv.rearrange("(c p) n -> p c n", p=128))
            wp_sb = consts.tile([LF, E], BF16, tag="wp")
            nc.sync.dma_start(out=wp_sb, in_=wp[:, :])
            bq_sb = consts.tile([LF, 1], F32, tag="bq")
            bk_sb = consts.tile([LF, 1], F32, tag="bk")
            bv_sb = consts.tile([LF, 1], F32, tag="bv")
            nc.sync.dma_start(out=bq_sb, in_=bq[:, :])
            nc.sync.dma_start(out=bk_sb, in_=bk[:, :])
            nc.sync.dma_start(out=bv_sb, in_=bv[:, :])
            ident = consts.tile([128, 128], BF16, tag="ident")
            make_identity(nc, ident)

            xT_r = xT.rearrange("(c p) s -> p c s", p=128)

            # per-batch state, filled lazily as phases are emitted
            qTs, kTs, vTs, vaugs, aoTs, u_alls, xts = {}, {}, {}, {}, {}, {}, {}

            def emit_A_group(b, sh, which):
                """One (s-half, tensor) block of the qkv projection."""
                if b not in qTs:
                    qTs[b] = acts.tile([128, S], BF16, tag="qT", name=f"qT{b}")
                    kTs[b] = acts.tile([128, S], BF16, tag="kT", name=f"kT{b}")
                    vTs[b] = vtp.tile([128, S], BF16, tag="vT", name=f"vT{b}")
                dst, w_sb, b_sb = {
                    "q": (qTs[b], wq_sb, bq_sb),
                    "k": (kTs[b], wk_sb, bk_sb),
                    "v": (vTs[b], wv_sb, bv_sb),
                }[which]
                if (b, sh) not in xts:
                    xt_new = xpool.tile(
                        [128, NCHUNK, SH], BF16, tag="xt", name=f"xt{b}{sh}"
                    )
                    s0 = b * S + sh * SH
                    nc.sync.dma_start(out=xt_new, in_=xT_r[:, :, s0 : s0 + SH])
                    xts[(b, sh)] = xt_new
                xt_sb = xts[(b, sh)]
                pss = []
                for st in range(NST):
                    ps = psA.tile([128, ST], F32, tag="psA", name=f"ps{st}")
                    pss.append(ps)
                for c in range(NCHUNK):
                    for st in range(NST):
                        mm(
                            pss[st],
                            lhsT=w_sb[:, c, :],
                            rhs=xt_sb[:, c, st * ST : (st + 1) * ST],
                            start=(c == 0),
                            stop=(c == NCHUNK - 1),
                        )
                for st in range(NST):
                    g0 = sh * SH + st * ST
                    nc.vector.tensor_scalar_add(dst[:, g0 : g0 + ST], pss[st], b_sb)

            def emit_transposes(b):
                """vT -> v_aug [t, (v_h | ones)] via PE transpose."""
                v_aug = vap.tile(
                    [128, NTT, 2 * (DH + 1)], BF16, tag="vaug", name=f"vaug{b}"
                )
                vaugs[b] = v_aug
                ones_col = ones16_d[:, 0:NTT].unsqueeze(2)
                nc.sync.dma_start(out=v_aug[:, :, DH : DH + 1], in_=ones_col)
                nc.sync.dma_start(
                    out=v_aug[:, :, 2 * DH + 1 : 2 * DH + 2], in_=ones_col
                )
                vT = vTs[b]
                for tt in range(NTT):
                    for h in range(HPC):
                        pst = psO.tile([128, ST], BF16, tag="psO", name="pst")
                        nc.tensor.matmul(
                            pst[:, 0:DH],
                            lhsT=vT[h * DH : (h + 1) * DH, tt * 128 : (tt + 1) * 128],
                            rhs=ident[h * DH : (h + 1) * DH, h * DH : (h + 1) * DH],
                            is_transpose=True,
                        )
                        nc.vector.tensor_copy(
                            v_aug[:, tt, h * (DH + 1) : h * (DH + 1) + DH],
                            pst[:, 0:DH],
                        )

            def emit_attention_qt(b, qt):
                """Attention + normalization + output projection for one
                512-wide q-tile."""
                if b not in aoTs:
                    aoTs[b] = acts.tile([128, S], BF16, tag="aoT", name=f"aoT{b}")
                    u_alls[b] = npool.tile(
                        [DH, HPC * NQ, ST], F32, tag="u_all", name=f"u_all{b}"
                    )
                qT, kT, v_aug, aoT = qTs[b], kTs[b], vaugs[b], aoTs[b]
                u_all = u_alls[b]
                qsl = slice(qt * ST, (qt + 1) * ST)
                out_ps = []
                for h in range(HPC):
                    o_ps = psO.tile([128, ST], F32, tag="psO", name=f"psO_{h}")
                    out_ps.append(o_ps)
                for tt in range(NTT):
                    tsl = slice(tt * 128, (tt + 1) * 128)
                    ps_s = psS.tile([128, HPC * ST], F32, tag="psS")
                    a = attp.tile([128, HPC * ST], BF16, tag="att")
                    for h in range(HPC):
                        hsl = slice(h * DH, (h + 1) * DH)
                        mm(
                            ps_s[:, h * ST : (h + 1) * ST],
                            lhsT=kT[hsl, tsl],
                            rhs=qT[hsl, qsl],
                            start=True,
                            stop=True,
                            tile_position=(h * DH, 0),
                        )
                    nc.scalar.activation(a, ps_s, mybir.ActivationFunctionType.Exp)
                    for h in range(HPC):
                        mm(
                            out_ps[h][0 : DH + 1, :],
                            lhsT=v_aug[:, tt, h * (DH + 1) : (h + 1) * (DH + 1)],
                            rhs=a[:, h * ST : (h + 1) * ST],
                            start=(tt == 0),
                            stop=(tt == NTT - 1),
                        )
                # normalize this q-tile (denominator row 64 of each psO)
                for h in range(HPC):
                    idx = qt * HPC + h
                    nc.vector.tensor_copy(u_all[:, idx, :], out_ps[h][0:DH, :])
                    rec = npool.tile([1, ST], F32, tag="rec")
                    nc.vector.reciprocal(rec, out_ps[h][DH : DH + 1, :])
                    bc_sb = npool.tile([DH, ST], F32, tag="bc")
                    nc.gpsimd.partition_broadcast(bc_sb, rec)
                    nc.vector.tensor_mul(
                        aoT[h * DH : (h + 1) * DH, qsl], u_all[:, idx, :], bc_sb
                    )
                # this q-tile's slice of the output projection
                for st in range(ST // 128):
                    s_loc = qt * ST + st * 128
                    r0 = b * S + s_loc
                    for eh in range(E // 512):
                        esl = slice(eh * 512, (eh + 1) * 512)
                        ps_y = psA.tile([128, 512], F32, tag="psA")
                        mm(
                            ps_y,
                            lhsT=aoT[:, s_loc : s_loc + 128],
                            rhs=wp_sb[:, esl],
                            start=True,
                            stop=True,
                        )
                        y_sb = ypool.tile([128, 512], F32, tag="y")
                        nc.vector.tensor_copy(y_sb, ps_y)
                        nc.sync.dma_start(out=y[r0 : r0 + 128, esl], in_=y_sb)

            # ---- emission schedule: batch 0's phase A, then per-q-tile
            # attention with the next batch's phase A interleaved ----
            INTERLEAVE = True
            for sh in range(2):
                for which in ("q", "k", "v"):
                    emit_A_group(0, sh, which)
            emit_transposes(0)
            items = [
                ("A", 0, "q"),
                ("A", 0, "k"),
                ("A", 0, "v"),
                ("A", 1, "v"),
                ("A", 1, "q"),
                ("A", 1, "k"),
            ]
            per_qt = -(-len(items) // NQ)  # ceil
            interleave = {
                qt: items[qt * per_qt : (qt + 1) * per_qt] for qt in range(NQ)
            }
            for b in range(B):
                if not INTERLEAVE and b > 0:
                    for sh in range(2):
                        for which in ("q", "k", "v"):
                            emit_A_group(b, sh, which)
                    emit_transposes(b)
                for qt in range(NQ):
                    emit_attention_qt(b, qt)
                    if INTERLEAVE and b + 1 < B:
                        for item in interleave.get(qt, []):
                            if item[0] == "A":
                                emit_A_group(b + 1, item[1], item[2])
                            else:
                                emit_transposes(b + 1)
                if INTERLEAVE and b + 1 < B:
                    emit_transposes(b + 1)

    nc.compile()
    return nc


_NC_CACHE = {}


def _get_nc(B, S):
    key = (B, S)
    if key not in _NC_CACHE:
        _NC_CACHE[key] = build_nc(B, S)
    return _NC_CACHE[key]


def make_in_maps(x, w_qkv, b_qkv, w_proj):
    B, S, _ = x.shape
    scale = DH**-0.5
    xT = np.ascontiguousarray(x.reshape(B * S, E).T).astype(NPBF16)
    in_maps = []
    for c in range(NCORES):
        cols = slice(c * LF, (c + 1) * LF)
        in_maps.append(
            {
                "xT": xT,
                "wq": (
                    np.ascontiguousarray(w_qkv[:, 0 * E : 1 * E][:, cols]) * scale
                ).astype(NPBF16),
                "wk": np.ascontiguousarray(w_qkv[:, 1 * E : 2 * E][:, cols]).astype(
                    NPBF16
                ),
                "wv": np.ascontiguousarray(w_qkv[:, 2 * E : 3 * E][:, cols]).astype(
                    NPBF16
                ),
                "bq": (b_qkv[0 * E : 1 * E][cols] * scale)
                .reshape(LF, 1)
                .astype(np.float32),
                "bk": b_qkv[1 * E : 2 * E][cols]
                .reshape(LF, 1)
                .astype(np.float32)
                .copy(),
                "bv": b_qkv[2 * E : 3 * E][cols]
                .reshape(LF, 1)
                .astype(np.float32)
                .copy(),
                "wp": np.ascontiguousarray(w_proj[cols, :]).astype(NPBF16),
                "ones16": np.ones((128, DH), dtype=NPBF16),
            }
        )
    return in_maps


def kernel_run(x, w_qkv, b_qkv, w_proj, b_proj, trace=False):
    x = np.asarray(x, dtype=np.float32)
    w_qkv = np.asarray(w_qkv, dtype=np.float32)
    b_qkv = np.asarray(b_qkv, dtype=np.float32)
    w_proj = np.asarray(w_proj, dtype=np.float32)
    b_proj = np.asarray(b_proj, dtype=np.float32)
    B, S, _ = x.shape
    nc = _get_nc(B, S)
    in_maps = make_in_maps(x, w_qkv, b_qkv, w_proj)
    res = run_bass_kernel_spmd(
        nc, in_maps, core_ids=list(range(NCORES)), trace=trace
    )
    y = res.results[0]["y"].astype(np.float64)
    for c in range(1, NCORES):
        y += res.results[c]["y"]
    y += b_proj[None, :]
    return y.astype(np.float32).reshape(B, S, E), res


def kernel(x, w_qkv, b_qkv, w_proj, b_proj):
    y, _ = kernel_run(x, w_qkv, b_qkv, w_proj, b_proj)
    return y
